# revision 1
# baseline (speedup 1.0000x reference)
"""Channel-attention (XCA-style) Trainium2 kernel, 8-way SPMD.

Shapes (hardcoded): B=4, N=16384, D=256, H=2 heads, c=128.
Sharding: core ci -> batch b=ci//2, token half ci%2 (T=8192 tokens/core).
The q@k^T contraction reduces over N, so each pair of cores all-reduces the
small per-head Gram matrices plus correction side-products and L2 norm pieces.

Per-core pipeline (fp32 I/O, bf16 matmul operands):
  - SWDGE cast-DMA loads raw token-major chunks (fp32->bf16)
  - LN is never materialized: folded into weights (host) + per-token scalars;
    centering/bias corrections are applied to the c x c Grams analytically
  - xbar DMA-transpose -> d-major tiles; PE matmuls; Gram + thin-row
    accumulation; pair AllReduce overlapped with the v matmul pass
  - softmax on 128x128 logits; attn @ v; Wo matmul; all v/bias corrections
    folded into the final evacuation
"""
import sys, types

sys.path.insert(0, "/opt/trn_rl_repo")

try:
    import antenv
    if "antenv.axon_hooks" not in sys.modules:
        _hooks = types.ModuleType("antenv.axon_hooks")
        _hooks._hook = None
        _hooks.set_axon_ntff_profile_hook = lambda h: setattr(_hooks, "_hook", h)
        _hooks.get_axon_ntff_profile_hook = lambda: _hooks._hook
        sys.modules["antenv.axon_hooks"] = _hooks
        antenv.axon_hooks = _hooks
        from trn_agent_boot.trn_boot import _ntff_profile_via_ctypes
        _hooks.set_axon_ntff_profile_hook(
            _ntff_profile_via_ctypes("/opt/axon/libaxon_pjrt.so"))
except Exception:
    pass

import numpy as np
import ml_dtypes

import concourse.bass as bass
import concourse.bacc as bacc
import concourse.mybir as mybir
import concourse.tile as tile
from concourse.bass_utils import run_bass_kernel_spmd

BF16 = ml_dtypes.bfloat16
F32 = mybir.dt.float32
BF = mybir.dt.bfloat16
AL = mybir.AluOpType
AF = mybir.ActivationFunctionType
AX = mybir.AxisListType

B, N, D, H = 4, 16384, 256, 2
C = D // H
T = N // 2
NT = T // 128              # 64 token tiles / core
EPS_LN = 1e-5
EPS_NORM = 1e-12
N_CORES = 8
CHT = 8                    # token tiles per chunk
NCH = NT // CHT            # 4 chunks
PACKW = 784                # collective pack width

_nc_cache = {}


def _bcast(ap, rows=128):
    """Broadcast (partition-step-0) a single-row AP across `rows` partitions."""
    return bass.AP(tensor=ap.tensor, offset=ap.offset,
                   ap=[[0, rows]] + [list(x) for x in ap.ap[1:]])


def _build_nc():
    nc = bacc.Bacc("TRN2", target_bir_lowering=False, debug=False,
                   num_devices=N_CORES)

    def ein(name, shape, dt=F32):
        return nc.dram_tensor(name, list(shape), dt, kind="ExternalInput")

    d_x = ein("x_r", [T, D])            # kv source shard (input_R)
    d_y = ein("x_s", [T, D])            # q source shard (input_S)
    d_wq = ein("wqT", [D, D], BF)       # [d, o] LN-folded
    d_wk = ein("wkT", [D, D], BF)
    d_wv = ein("wvT", [D, D], BF)
    d_wo = ein("woT", [D, D], BF)
    d_svc = ein("sv_col", [D, 1], BF)
    d_bvc = ein("bv2_col", [D, 1], BF)
    d_sqc = ein("sq_col", [128, H])
    d_bqc = ein("bq2_col", [128, H])
    d_skc = ein("sk_col", [128, H])
    d_bkc = ein("bk2_col", [128, H])
    d_skr = ein("sk_row", [1, D])
    d_bkr = ein("bk2_row", [1, D])
    d_bor = ein("bo_row", [1, D])
    d_eye = ein("eye", [128, 128])
    d_temp = ein("temp", [1, H])
    d_out = nc.dram_tensor("out", [T, D], F32, kind="ExternalOutput")

    xv = d_x.rearrange("(j p) d -> p j d", p=128)
    yv = d_y.rearrange("(j p) d -> p j d", p=128)
    outv = d_out.rearrange("(j p) d -> p j d", p=128)

    with tile.TileContext(nc) as tc:
        import contextlib
        with contextlib.ExitStack() as ctx:
            _body(ctx, tc, nc, xv, yv, outv, d_wq, d_wk, d_wv, d_wo,
                  d_svc, d_bvc, d_sqc, d_bqc, d_skc, d_bkc, d_skr, d_bkr,
                  d_bor, d_eye, d_temp)
    nc.finalize()
    return nc


def _body(ctx, tc, nc, xv, yv, outv, d_wq, d_wk, d_wv, d_wo, d_svc, d_bvc,
          d_sqc, d_bqc, d_skc, d_bkc, d_skr, d_bkr, d_bor, d_eye, d_temp):
    E = ctx.enter_context
    consts = E(tc.tile_pool(name="consts", bufs=1))
    stats = E(tc.tile_pool(name="stats", bufs=1))
    stage = E(tc.tile_pool(name="stage", bufs=2))
    xtp = E(tc.tile_pool(name="xtp", bufs=2))
    qkp = E(tc.tile_pool(name="qkp", bufs=2))
    pers = E(tc.tile_pool(name="pers", bufs=1))
    post = E(tc.tile_pool(name="post", bufs=1))
    small = E(tc.tile_pool(name="small", bufs=4))
    outp = E(tc.tile_pool(name="outp", bufs=2))
    dram = E(tc.tile_pool(name="dram", bufs=1, space="DRAM"))
    accps = E(tc.tile_pool(name="accps", bufs=1, space="PSUM"))

    # ---------------- constants ----------------
    wq_sb = consts.tile([128, 2, D], BF, tag="wq")
    wk_sb = consts.tile([128, 2, D], BF, tag="wk")
    wv_sb = consts.tile([128, 2, D], BF, tag="wv")
    wo_sb = consts.tile([128, 2, D], BF, tag="wo")
    for dst, src in ((wq_sb, d_wq), (wk_sb, d_wk), (wv_sb, d_wv), (wo_sb, d_wo)):
        nc.sync.dma_start(out=dst[:], in_=src.rearrange("(h p) o -> p h o", p=128))
    sv_col = consts.tile([128, 2, 1], BF, tag="svc")
    bv_col = consts.tile([128, 2, 1], BF, tag="bvc")
    nc.sync.dma_start(out=sv_col[:], in_=d_svc.rearrange("(h p) o -> p h o", p=128))
    nc.sync.dma_start(out=bv_col[:], in_=d_bvc.rearrange("(h p) o -> p h o", p=128))
    sq_col = consts.tile([128, H], F32, tag="sqc")
    bq_col = consts.tile([128, H], F32, tag="bqc")
    sk_col = consts.tile([128, H], F32, tag="skc")
    bk_col = consts.tile([128, H], F32, tag="bkc")
    for dst, src in ((sq_col, d_sqc), (bq_col, d_bqc), (sk_col, d_skc),
                     (bk_col, d_bkc)):
        nc.sync.dma_start(out=dst[:], in_=src[:, :])
    skr_b = consts.tile([128, D], F32, tag="skrb")
    bkr_b = consts.tile([128, D], F32, tag="bkrb")
    nc.sync.dma_start(out=skr_b[:], in_=_bcast(d_skr[:, :]))
    nc.sync.dma_start(out=bkr_b[:], in_=_bcast(d_bkr[:, :]))
    bo_row = consts.tile([1, D], F32, tag="bor")
    nc.sync.dma_start(out=bo_row[:], in_=d_bor[:, :])
    eye_sb = consts.tile([128, 128], F32, tag="eye")
    nc.sync.dma_start(out=eye_sb[:], in_=d_eye[:, :])
    temp_b = consts.tile([128, H], F32, tag="tempb")
    nc.sync.dma_start(out=temp_b[:], in_=_bcast(d_temp[:, :]))
    ones_bf = consts.tile([128, 1], BF, tag="ones")
    nc.vector.memset(ones_bf[:], 1.0)
    epsln = consts.tile([128, 1], F32, tag="epsln")
    nc.vector.memset(epsln[:], EPS_LN)
    zb = consts.tile([128, 1], F32, tag="zb")
    nc.vector.memset(zb[:], 0.0)

    # ---------------- stats state ----------------
    ssq_r = stats.tile([128, NT], F32, tag="ssq_r")
    ssq_s = stats.tile([128, NT], F32, tag="ssq_s")
    invs_r = stats.tile([128, NT], F32, tag="invs_r")
    invs_s = stats.tile([128, NT], F32, tag="invs_s")
    arn = stats.tile([128, NT], F32, tag="arn")        # -aR = -muR*invsR (f32)
    wcols = stats.tile([128, NT, 3], BF, tag="wcols")  # [-aS, -aR, 1]

    nc.vector.memset(wcols[:, :, 2], 1.0)
    sq_scr = stats.tile([128, 256], F32, tag="sq_scr")

    xtr_all = pers.tile([128, NT, 2, 128], BF, tag="xtr")
    vt_all = pers.tile([128, 2, T], BF, tag="vt")

    acc = accps.tile([128, 1024], F32, tag="acc")
    # acc cols: Gt h0 0:128 h1 128:256 | Hqq 256:512 | Hkk 512:768
    # thinQ [0:3, 768:1024] thinK [4:7, 768:1024] Sc [8:11, 768:771]

    # ================= phase 1: stream chunks =================
    with tc.tile_pool(name="qkps", bufs=2, space="PSUM") as qkps, \
         tc.tile_pool(name="sumps", bufs=2, space="PSUM") as sumps:
        for ch in range(NCH):
            j0 = ch * CHT
            mu_rows = stage.tile([16, CHT * 128], BF, tag="mu_rows")
            mus_row = stage.tile([1, CHT * 128], BF, tag="mus_row")
            nc.gpsimd.memset(mu_rows[:, :], 0.0)
            xr_tm = stage.tile([128, CHT, D], BF, tag="xr_tm")
            ys_tm = stage.tile([128, CHT, D], BF, tag="ys_tm")
            nc.gpsimd.dma_start(out=xr_tm[:], in_=xv[:, j0:j0 + CHT, :])
            nc.gpsimd.dma_start(out=ys_tm[:], in_=yv[:, j0:j0 + CHT, :])

            for jj in range(CHT):
                j = j0 + jj
                nc.vector.scalar_tensor_tensor(
                    out=sq_scr[:], in0=xr_tm[:, jj, :], scalar=0.0,
                    op0=AL.bypass, op1=AL.mult, in1=xr_tm[:, jj, :],
                    accum_out=ssq_r[:, j:j + 1])
                nc.vector.scalar_tensor_tensor(
                    out=sq_scr[:], in0=ys_tm[:, jj, :], scalar=0.0,
                    op0=AL.bypass, op1=AL.mult, in1=ys_tm[:, jj, :],
                    accum_out=ssq_s[:, j:j + 1])

            # d-major transposes (xbar): out[p, e, t] = in[t, e*128+p]
            nc.sync.dma_start_transpose(xtr_all[:, j0:j0 + CHT, :, :], xr_tm[:])
            ytr = xtp.tile([128, CHT, 2, 128], BF, tag="ytr")
            nc.sync.dma_start_transpose(ytr[:], ys_tm[:])

            # means via PE ones-matmuls (rows 0=R, 1=S), 512-token groups
            for g in range(CHT // 4):
                sps = sumps.tile([1, 1024], F32, tag="sums")
                for q4 in range(4):
                    jj = g * 4 + q4
                    for hh in range(2):
                        nc.tensor.matmul(
                            out=sps[0:1, q4 * 128:(q4 + 1) * 128],
                            lhsT=ones_bf[:], rhs=xtr_all[:, j0 + jj, hh, :],
                            start=(hh == 0), stop=(hh == 1))
                        nc.tensor.matmul(
                            out=sps[0:1, 512 + q4 * 128:512 + (q4 + 1) * 128],
                            lhsT=ones_bf[:], rhs=ytr[:, jj, hh, :],
                            start=(hh == 0), stop=(hh == 1))
                t0 = g * 4 * 128
                nc.scalar.activation(out=mu_rows[0:1, t0:t0 + 512],
                                     in_=sps[0:1, 0:512],
                                     func=AF.Copy, bias=0.0, scale=1.0 / D)
                nc.scalar.activation(out=mus_row[0:1, t0:t0 + 512],
                                     in_=sps[0:1, 512:1024],
                                     func=AF.Copy, bias=0.0, scale=1.0 / D)

            # stats to partition layout via xbar of the mu_rows chunk
            nc.sync.dma_start(out=mu_rows[1:2, :], in_=mus_row[0:1, :])
            mu_part = small.tile([128, CHT, 16], BF, tag="mu_part")
            nc.sync.dma_start_transpose(mu_part[:], mu_rows[:, :])

            for inp, (ssq, invs, wslot) in enumerate(
                    ((ssq_r, invs_r, 1), (ssq_s, invs_s, 0))):
                mu = small.tile([128, CHT], F32, tag="mu_f")
                nc.vector.tensor_scalar(mu[:], mu_part[:, :, inp], 1.0, None,
                                        AL.mult)
                var = small.tile([128, CHT], F32, tag="var")
                nc.vector.scalar_tensor_tensor(
                    out=var[:], in0=mu[:], scalar=-1.0, op0=AL.mult,
                    op1=AL.mult, in1=mu[:])
                nc.vector.scalar_tensor_tensor(
                    out=var[:], in0=ssq[:, j0:j0 + CHT], scalar=1.0 / D,
                    op0=AL.mult, op1=AL.add, in1=var[:])
                sig = small.tile([128, CHT], F32, tag="sig")
                nc.scalar.activation(out=sig[:], in_=var[:], func=AF.Sqrt,
                                     bias=epsln[:, :], scale=1.0)
                nc.vector.reciprocal(out=invs[:, j0:j0 + CHT], in_=sig[:])
                nc.vector.scalar_tensor_tensor(
                    out=wcols[:, j0:j0 + CHT, wslot], in0=mu[:], scalar=-1.0,
                    op0=AL.mult, op1=AL.mult, in1=invs[:, j0:j0 + CHT])
                if inp == 0:
                    nc.vector.scalar_tensor_tensor(
                        out=arn[:, j0:j0 + CHT], in0=mu[:], scalar=-1.0,
                        op0=AL.mult, op1=AL.mult, in1=invs[:, j0:j0 + CHT])

            # q/k matmuls + evac + gram accumulation
            qt_c = qkp.tile([128, CHT, D], BF, tag="qt")
            kt_c = qkp.tile([128, CHT, D], BF, tag="kt")
            for jj in range(CHT):
                j = j0 + jj
                qkt = qkps.tile([128, 512], F32, tag="qk")
                qps = qkt[:, 0:256]
                kps = qkt[:, 256:512]
                for hh in range(2):
                    nc.tensor.matmul(out=qps, lhsT=ytr[:, jj, hh, :],
                                     rhs=wq_sb[:, hh, :],
                                     start=(hh == 0), stop=(hh == 1))
                    nc.tensor.matmul(out=kps, lhsT=xtr_all[:, j, hh, :],
                                     rhs=wk_sb[:, hh, :],
                                     start=(hh == 0), stop=(hh == 1))
                nc.vector.tensor_scalar(qt_c[:, jj, :], qps,
                                        invs_s[:, j:j + 1], None, AL.mult)
                nc.scalar.activation(out=kt_c[:, jj, :], in_=kps,
                                     func=AF.Copy, bias=0.0,
                                     scale=invs_r[:, j:j + 1])
                st = (j == 0)
                sp = (j == NT - 1)
                for hh in range(2):
                    qs = qt_c[:, jj, hh * 128:(hh + 1) * 128]
                    ks = kt_c[:, jj, hh * 128:(hh + 1) * 128]
                    nc.tensor.matmul(out=acc[:, hh * 128:(hh + 1) * 128],
                                     lhsT=qs, rhs=ks, start=st, stop=sp)
                    nc.tensor.matmul(
                        out=acc[:, 256 + hh * 128:256 + (hh + 1) * 128],
                        lhsT=qs, rhs=qs, start=st, stop=sp)
                    nc.tensor.matmul(
                        out=acc[:, 512 + hh * 128:512 + (hh + 1) * 128],
                        lhsT=ks, rhs=ks, start=st, stop=sp)
                wc = wcols[:, j, :]
                nc.tensor.matmul(out=acc[0:3, 768:1024], lhsT=wc,
                                 rhs=qt_c[:, jj, :], start=st, stop=sp)
                nc.tensor.matmul(out=acc[32:35, 768:1024], lhsT=wc,
                                 rhs=kt_c[:, jj, :], start=st, stop=sp)
                nc.tensor.matmul(out=acc[64:67, 768:771], lhsT=wc, rhs=wc,
                                 start=st, stop=sp)

    # ================= phase 2: pack + collective =================
    gt_sb = post.tile([128, 256], F32, tag="gt")
    nc.vector.tensor_scalar(gt_sb[:], acc[:, 0:256], 1.0, None, AL.mult)
    dq_sb = post.tile([128, H], F32, tag="dq")
    dk_sb = post.tile([128, H], F32, tag="dk")
    dscr = post.tile([128, 128], F32, tag="dscr")
    for hh in range(2):
        nc.vector.scalar_tensor_tensor(
            out=dscr[:], in0=acc[:, 256 + hh * 128:256 + (hh + 1) * 128],
            scalar=1.0, op0=AL.mult, op1=AL.mult, in1=eye_sb[:],
            accum_out=dq_sb[:, hh:hh + 1])
        nc.vector.scalar_tensor_tensor(
            out=dscr[:], in0=acc[:, 512 + hh * 128:512 + (hh + 1) * 128],
            scalar=1.0, op0=AL.mult, op1=AL.mult, in1=eye_sb[:],
            accum_out=dk_sb[:, hh:hh + 1])
    tq_sb = post.tile([3, 256], F32, tag="tq")
    tk_sb = post.tile([3, 256], F32, tag="tk")
    sc_sb = post.tile([3, 3], F32, tag="sc")
    nc.vector.tensor_scalar(tq_sb[:], acc[0:3, 768:1024], 1.0, None, AL.mult)
    nc.vector.tensor_scalar(tk_sb[:], acc[32:35, 768:1024], 1.0, None, AL.mult)
    nc.vector.tensor_scalar(sc_sb[:], acc[64:67, 768:771], 1.0, None, AL.mult)

    cc_in = dram.tile([128, PACKW], F32)
    cc_out = dram.tile([128, PACKW], F32)
    nc.gpsimd.dma_start(out=cc_in[:, 0:256], in_=gt_sb[:])
    nc.gpsimd.dma_start(out=cc_in[:, 256:258], in_=dq_sb[:])
    nc.gpsimd.dma_start(out=cc_in[:, 258:260], in_=dk_sb[:])
    nc.gpsimd.dma_start(out=cc_in[0:3, 260:516], in_=tq_sb[:])
    nc.gpsimd.dma_start(out=cc_in[0:3, 516:772], in_=tk_sb[:])
    nc.gpsimd.dma_start(out=cc_in[0:3, 772:775], in_=sc_sb[:])
    nc.gpsimd.collective_compute(
        "AllReduce", AL.add,
        replica_groups=[[0, 1], [2, 3], [4, 5], [6, 7]],
        ins=[cc_in.opt()], outs=[cc_out.opt()])

    with tc.tile_pool(name="mmps", bufs=2, space="PSUM") as mmps:
        # ---- v matmuls (no dependency on the collective -> overlaps it) ----
        for g in range(T // 512):
            vps = mmps.tile([128, 2, 512], F32, tag="mm")
            for hh in range(2):
                for dh in range(2):
                    nc.tensor.matmul(
                        out=vps[:, hh, :],
                        lhsT=wv_sb[:, dh, hh * 128:(hh + 1) * 128],
                        rhs=xtr_all[:, g * 4:(g + 1) * 4, dh, :],
                        start=(dh == 0), stop=(dh == 1))
            for hh in range(2):
                nc.scalar.activation(
                    out=vt_all[:, hh, g * 512:(g + 1) * 512],
                    in_=vps[:, hh, :], func=AF.Copy, bias=0.0, scale=1.0)

        # ================= phase 3: post-collective assembly ================
        red = post.tile([128, PACKW], F32, tag="red")
        nc.gpsimd.dma_start(out=red[:], in_=cc_out[:, :])
        rG = red[:, 0:256]
        rDq = red[:, 256:258]
        rDk = red[:, 258:260]

        # thin rows -> DRAM bounce; read back transposed / broadcast (f32)
        thin_d = dram.tile([6, 256], F32)
        nc.gpsimd.dma_start(out=thin_d[0:3, :], in_=red[0:3, 260:516])
        nc.gpsimd.dma_start(out=thin_d[3:6, :], in_=red[0:3, 516:772])
        sc_d = dram.tile([3, 3], F32)
        nc.gpsimd.dma_start(out=sc_d[:, :], in_=red[0:3, 772:775])

        # tcols[p, h, s] = thin row s at channel c=p of head h
        tcols = post.tile([128, H, 6], F32, tag="tcols")
        tap = thin_d[:, :]
        for hh in range(2):
            nc.sync.dma_start(out=tcols[:, hh, :], in_=bass.AP(
                tensor=tap.tensor, offset=tap.offset + hh * 128,
                ap=[[1, 128], [256, 6]]))
        # row broadcasts of RkA (row 3) and Rk0 (row 5)
        row3 = post.tile([128, 256], F32, tag="row3")
        row4 = post.tile([128, 256], F32, tag="row4")
        nc.sync.dma_start(out=row3[:], in_=_bcast(thin_d[3:4, :]))
        nc.sync.dma_start(out=row4[:], in_=_bcast(thin_d[5:6, :]))
        sAA = small.tile([128, 1], F32, tag="sAA")
        sAB = small.tile([128, 1], F32, tag="sAB")
        sA = small.tile([128, 1], F32, tag="sA")
        sBB = small.tile([128, 1], F32, tag="sBB")
        sB = small.tile([128, 1], F32, tag="sB")
        for dst, (r, c) in ((sAA, (0, 0)), (sAB, (0, 1)), (sA, (0, 2)),
                            (sBB, (1, 1)), (sB, (1, 2))):
            nc.sync.dma_start(out=dst[:], in_=_bcast(sc_d[r:r + 1, c:c + 1]))

        # row3 += s_k*Sab + bk2*Sa ; row4 += s_k*Sb + bk2*T
        t_r = post.tile([128, 256], F32, tag="t_r")
        nc.vector.scalar_tensor_tensor(out=t_r[:], in0=skr_b[:],
                                       scalar=sAB[:, :], op0=AL.mult,
                                       op1=AL.add, in1=row3[:])
        nc.vector.scalar_tensor_tensor(out=row3[:], in0=bkr_b[:],
                                       scalar=sA[:, :], op0=AL.mult,
                                       op1=AL.add, in1=t_r[:])
        nc.vector.scalar_tensor_tensor(out=t_r[:], in0=skr_b[:],
                                       scalar=sB[:, :], op0=AL.mult,
                                       op1=AL.add, in1=row4[:])
        nc.vector.scalar_tensor_tensor(out=row4[:], in0=bkr_b[:],
                                       scalar=float(N), op0=AL.mult,
                                       op1=AL.add, in1=t_r[:])

        # G assembly per head (in place on rG)
        for hh in range(2):
            Gh = rG[:, hh * 128:(hh + 1) * 128]
            nc.vector.scalar_tensor_tensor(
                out=Gh, in0=skr_b[:, hh * 128:(hh + 1) * 128],
                scalar=tcols[:, hh, 1:2], op0=AL.mult, op1=AL.add, in1=Gh)
            nc.vector.scalar_tensor_tensor(
                out=Gh, in0=bkr_b[:, hh * 128:(hh + 1) * 128],
                scalar=tcols[:, hh, 2:3], op0=AL.mult, op1=AL.add, in1=Gh)
            nc.vector.scalar_tensor_tensor(
                out=Gh, in0=row3[:, hh * 128:(hh + 1) * 128],
                scalar=sq_col[:, hh:hh + 1], op0=AL.mult, op1=AL.add, in1=Gh)
            nc.vector.scalar_tensor_tensor(
                out=Gh, in0=row4[:, hh * 128:(hh + 1) * 128],
                scalar=bq_col[:, hh:hh + 1], op0=AL.mult, op1=AL.add, in1=Gh)

        # norms
        def _norm2(dst, dvec, ucol, gcol, cA, c0, sXX, sX):
            t1 = small.tile([128, H], F32, tag="n_t1")
            nc.vector.tensor_tensor(out=t1[:], in0=ucol, in1=cA, op=AL.mult)
            nc.vector.scalar_tensor_tensor(out=dst[:], in0=t1[:], scalar=2.0,
                                           op0=AL.mult, op1=AL.add, in1=dvec)
            nc.vector.tensor_tensor(out=t1[:], in0=gcol, in1=c0, op=AL.mult)
            nc.vector.scalar_tensor_tensor(out=dst[:], in0=t1[:], scalar=2.0,
                                           op0=AL.mult, op1=AL.add, in1=dst[:])
            nc.vector.tensor_tensor(out=t1[:], in0=ucol, in1=ucol, op=AL.mult)
            nc.vector.scalar_tensor_tensor(out=dst[:], in0=t1[:],
                                           scalar=sXX[:, :], op0=AL.mult,
                                           op1=AL.add, in1=dst[:])
            nc.vector.tensor_tensor(out=t1[:], in0=ucol, in1=gcol, op=AL.mult)
            nc.vector.tensor_scalar(t1[:], t1[:], 2.0, None, AL.mult)
            nc.vector.scalar_tensor_tensor(out=dst[:], in0=t1[:],
                                           scalar=sX[:, :], op0=AL.mult,
                                           op1=AL.add, in1=dst[:])
            nc.vector.tensor_tensor(out=t1[:], in0=gcol, in1=gcol, op=AL.mult)
            nc.vector.scalar_tensor_tensor(out=dst[:], in0=t1[:],
                                           scalar=float(N), op0=AL.mult,
                                           op1=AL.add, in1=dst[:])

        qn2 = small.tile([128, H], F32, tag="qn2")
        kn2 = small.tile([128, H], F32, tag="kn2")
        _norm2(qn2, rDq, sq_col[:, :], bq_col[:, :], tcols[:, :, 0],
               tcols[:, :, 2], sAA, sA)
        _norm2(kn2, rDk, sk_col[:, :], bk_col[:, :], tcols[:, :, 4],
               tcols[:, :, 5], sBB, sB)

        def _invnorm(dst, src, mul_temp):
            sq = small.tile([128, H], F32, tag="invn_sq")
            nc.scalar.activation(out=sq[:], in_=src[:], func=AF.Sqrt,
                                 bias=zb[:, :], scale=1.0)
            nc.vector.tensor_scalar_max(sq[:], sq[:], EPS_NORM)
            nc.vector.reciprocal(out=dst[:], in_=sq[:])
            if mul_temp:
                nc.vector.tensor_tensor(out=dst[:], in0=dst[:],
                                        in1=temp_b[:, 0:H], op=AL.mult)

        invq = small.tile([128, H], F32, tag="invq")
        invk = small.tile([128, H], F32, tag="invk")
        _invnorm(invq, qn2, True)
        _invnorm(invk, kn2, False)

        # invk column -> per-head broadcast rows (via DRAM bounce).
        # Write transposed ([2, 128] row-contiguous) so the broadcast read
        # generates 512B-contiguous runs, not a 4-byte gather storm.
        ik_d = dram.tile([2, 128], F32)
        ik_ap = ik_d[:, :]
        nc.gpsimd.dma_start(out=bass.AP(
            tensor=ik_ap.tensor, offset=ik_ap.offset,
            ap=[[1, 128], [128, 2]]), in_=invk[:])
        ikb = post.tile([128, 2, 128], F32, tag="ikb")
        for hh in range(2):
            nc.sync.dma_start(out=ikb[:, hh, :], in_=_bcast(ik_d[hh:hh + 1, :]))

        # softmax per head
        attn = post.tile([128, 2, 128], F32, tag="attn")
        for hh in range(2):
            Gh = rG[:, hh * 128:(hh + 1) * 128]
            nc.vector.tensor_scalar(Gh, Gh, invq[:, hh:hh + 1], None, AL.mult)
            nc.vector.tensor_tensor(out=Gh, in0=Gh, in1=ikb[:, hh, :],
                                    op=AL.mult)
            rmax = small.tile([128, 1], F32, tag="rmax")
            nc.vector.tensor_reduce(out=rmax[:], in_=Gh, op=AL.max, axis=AX.X)
            nc.vector.tensor_scalar(rmax[:], rmax[:], -1.0, None, AL.mult)
            nc.scalar.activation(out=attn[:, hh, :], in_=Gh, func=AF.Exp,
                                 bias=rmax[:, :], scale=1.0)
            rsum = small.tile([128, 1], F32, tag="rsum")
            nc.vector.tensor_reduce(out=rsum[:], in_=attn[:, hh, :], op=AL.add,
                                    axis=AX.X)
            nc.vector.reciprocal(out=rsum[:], in_=rsum[:])
            nc.vector.tensor_scalar(attn[:, hh, :], attn[:, hh, :],
                                    rsum[:, :], None, AL.mult)

        # attn^T via PE -> bf16
        attnT = post.tile([128, 2, 128], BF, tag="attnT")
        for hh in range(2):
            tp = mmps.tile([128, 2, 512], F32, tag="mm")
            nc.tensor.transpose(tp[:, 0, 0:128], attn[:, hh, :], eye_sb[:])
            nc.scalar.activation(out=attnT[:, hh, :], in_=tp[:, 0, 0:128],
                                 func=AF.Copy, bias=0.0, scale=1.0)

        # E vectors ([c, head, (E1,E0)]) and F rows
        e_sb = post.tile([128, 2, 2], BF, tag="e_sb")
        for hh in range(2):
            eps_mm = mmps.tile([128, 2, 512], F32, tag="mm")
            svbv = small.tile([128, 2], BF, tag="svbv")
            nc.vector.tensor_scalar(svbv[:, 0:1], sv_col[:, hh, :], 1.0, None,
                                    AL.mult)
            nc.vector.tensor_scalar(svbv[:, 1:2], bv_col[:, hh, :], 1.0, None,
                                    AL.mult)
            nc.tensor.matmul(out=eps_mm[:, 0, 0:2], lhsT=attnT[:, hh, :],
                             rhs=svbv[:], start=True, stop=True)
            nc.vector.tensor_scalar(e_sb[:, hh, :], eps_mm[:, 0, 0:2], 1.0,
                                    None, AL.mult)
        fps = mmps.tile([128, 2, 512], F32, tag="mm")
        for hh in range(2):
            nc.tensor.matmul(out=fps[0:1, 0, 0:256], lhsT=e_sb[:, hh, 0:1],
                             rhs=wo_sb[:, hh, :], start=(hh == 0),
                             stop=(hh == 1))
            nc.tensor.matmul(out=fps[32:33, 0, 0:256], lhsT=e_sb[:, hh, 1:2],
                             rhs=wo_sb[:, hh, :], start=(hh == 0),
                             stop=(hh == 1))
        f1_sb = post.tile([1, 256], F32, tag="f1_sb")
        f2_sb = post.tile([1, 256], F32, tag="f2_sb")
        nc.vector.tensor_scalar(f1_sb[:, :], fps[0:1, 0, 0:256], 1.0, None,
                                AL.mult)
        nc.vector.tensor_scalar(f2_sb[:, :], fps[32:33, 0, 0:256], 1.0, None,
                                AL.mult)
        nc.vector.tensor_tensor(out=f2_sb[:, :], in0=f2_sb[:, :],
                                in1=bo_row[:, :], op=AL.add)
        f_d = dram.tile([2, 256], F32)
        nc.gpsimd.dma_start(out=f_d[0:1, :], in_=f1_sb[:, :])
        nc.gpsimd.dma_start(out=f_d[1:2, :], in_=f2_sb[:, :])
        f1b = post.tile([128, 256], F32, tag="f1b")
        f2b = post.tile([128, 256], F32, tag="f2b")
        nc.sync.dma_start(out=f1b[:], in_=_bcast(f_d[0:1, :]))
        nc.sync.dma_start(out=f2b[:], in_=_bcast(f_d[1:2, :]))

        # ================= phase 4: av + wo + final evac =================
        for g in range(T // 512):
            avps = mmps.tile([128, 2, 512], F32, tag="mm")
            for hh in range(2):
                nc.tensor.matmul(out=avps[:, hh, :], lhsT=attnT[:, hh, :],
                                 rhs=vt_all[:, hh, g * 512:(g + 1) * 512],
                                 start=True, stop=True)
            av_sb = outp.tile([128, 2, 512], BF, tag="av_sb")
            for hh in range(2):
                nc.scalar.activation(out=av_sb[:, hh, :], in_=avps[:, hh, :],
                                     func=AF.Copy, bias=0.0, scale=1.0)
            ops = mmps.tile([128, 2, 512], F32, tag="mm")
            out_sb = outp.tile([128, 4, 256], F32, tag="out_sb")
            for q4 in range(4):
                j = g * 4 + q4
                for hh in range(2):
                    nc.tensor.matmul(
                        out=ops[:, q4 // 2, (q4 % 2) * 256:(q4 % 2 + 1) * 256],
                        lhsT=av_sb[:, hh, q4 * 128:(q4 + 1) * 128],
                        rhs=wo_sb[:, hh, :], start=(hh == 0), stop=(hh == 1))
                t1 = outp.tile([128, 256], F32, tag="t1")
                nc.vector.scalar_tensor_tensor(
                    out=t1[:], in0=f1b[:], scalar=arn[:, j:j + 1],
                    op0=AL.mult, op1=AL.add, in1=f2b[:])
                nc.vector.scalar_tensor_tensor(
                    out=out_sb[:, q4, :],
                    in0=ops[:, q4 // 2, (q4 % 2) * 256:(q4 % 2 + 1) * 256],
                    scalar=invs_r[:, j:j + 1], op0=AL.mult, op1=AL.add,
                    in1=t1[:])
            nc.sync.dma_start(out=outv[:, g * 4:(g + 1) * 4, :], in_=out_sb[:])


# ======================= host side =======================

def _prep_shared(inputs):
    f32 = np.float32
    Wq = np.asarray(inputs["Wq"], f32)
    bq = np.asarray(inputs["bq"], f32)
    Wkv = np.asarray(inputs["Wkv"], f32)
    bkv = np.asarray(inputs["bkv"], f32)
    Wo = np.asarray(inputs["Wo"], f32)
    bo = np.asarray(inputs["bo"], f32)
    lnS_w = np.asarray(inputs["lnS_w"], f32)
    lnS_b = np.asarray(inputs["lnS_b"], f32)
    lnR_w = np.asarray(inputs["lnR_w"], f32)
    lnR_b = np.asarray(inputs["lnR_b"], f32)
    temp = np.asarray(inputs["temperature"], f32).reshape(H)

    Wk, Wv = Wkv[:D], Wkv[D:]
    Wqp = Wq * lnS_w[None, :]
    Wkp = Wk * lnR_w[None, :]
    Wvp = Wv * lnR_w[None, :]
    bq2 = Wq @ lnS_b + bq
    bk2 = Wk @ lnR_b + bkv[:D]
    bv2 = Wv @ lnR_b + bkv[D:]
    s_q, s_k, s_v = Wqp.sum(1), Wkp.sum(1), Wvp.sum(1)

    def colh(v):
        return np.ascontiguousarray(v.reshape(H, 128).T, f32)

    return {
        "wqT": np.ascontiguousarray(Wqp.T).astype(BF16),
        "wkT": np.ascontiguousarray(Wkp.T).astype(BF16),
        "wvT": np.ascontiguousarray(Wvp.T).astype(BF16),
        "woT": np.ascontiguousarray(Wo.T).astype(BF16),
        "sv_col": s_v.reshape(D, 1).astype(BF16),
        "bv2_col": bv2.reshape(D, 1).astype(BF16),
        "sq_col": colh(s_q),
        "bq2_col": colh(bq2),
        "sk_col": colh(s_k),
        "bk2_col": colh(bk2),
        "sk_row": s_k.reshape(1, D).astype(f32),
        "bk2_row": bk2.reshape(1, D).astype(f32),
        "bo_row": bo.reshape(1, D).astype(f32),
        "eye": np.eye(128, dtype=f32),
        "temp": temp.reshape(1, H).astype(f32),
    }


def _get_nc():
    if "nc" not in _nc_cache:
        _nc_cache["nc"] = _build_nc()
    return _nc_cache["nc"]


def run(inputs, trace=False):
    nc = _get_nc()
    shared = _prep_shared(inputs)
    iR = np.asarray(inputs["input_R"], np.float32)
    iS = np.asarray(inputs["input_S"], np.float32)
    in_maps = []
    for ci in range(N_CORES):
        b, half = ci // 2, ci % 2
        m = dict(shared)
        m["x_r"] = np.ascontiguousarray(iR[b, half * T:(half + 1) * T])
        m["x_s"] = np.ascontiguousarray(iS[b, half * T:(half + 1) * T])
        in_maps.append(m)
    res = run_bass_kernel_spmd(nc, in_maps, list(range(N_CORES)), trace=trace)
    out = np.zeros((B, N, D), np.float32)
    for ci in range(N_CORES):
        b, half = ci // 2, ci % 2
        out[b, half * T:(half + 1) * T] = res.results[ci]["out"]
    return out, res


def kernel(**inputs):
    out, _ = run(inputs, trace=False)
    return out



# revision 9
# speedup vs baseline: 1.3512x; 1.3512x over previous
"""Channel-attention (XCA-style) Trainium2 kernel, 8-way SPMD — v2.

Shapes (hardcoded): B=4, N=16384, D=256, H=2 heads, c=128.
Sharding: core ci -> batch b=ci//2, token half ci%2 (T=8192 tokens/core).

Covariance formulation: per core, exactly LayerNorm the bf16 token tiles
(one fused scale+bias op per tile), accumulate three 256x256 token-
contracted Grams (M_SS, M_RS, M_RR) plus channel sums on the PE, then
  G    = Wq' M_SR Wk'^T + rank-1 bias outer-products   (head-diag blocks)
  dq/dk = diag(Wq' M_SS Wq'^T) + bias terms            (eye-dot on PE out)
One pair AllReduce of [128, 260] (G | dq | dk). Post-collective, softmax
gives attn; attn@v and the output projection collapse into a single
256x256 effective weight W_eff = Wo . blockdiag(attn_h) . Wv', applied to
the (transposed) normalized R in one matmul pass; per-token work in the
output phase is a single PSUM+bias-row evacuation. Output lands bf16 in
DRAM; the host upcasts to fp32.
"""
import sys, types

sys.path.insert(0, "/opt/trn_rl_repo")

try:
    import antenv
    if "antenv.axon_hooks" not in sys.modules:
        _hooks = types.ModuleType("antenv.axon_hooks")
        _hooks._hook = None
        _hooks.set_axon_ntff_profile_hook = lambda h: setattr(_hooks, "_hook", h)
        _hooks.get_axon_ntff_profile_hook = lambda: _hooks._hook
        sys.modules["antenv.axon_hooks"] = _hooks
        antenv.axon_hooks = _hooks
        from trn_agent_boot.trn_boot import _ntff_profile_via_ctypes
        _hooks.set_axon_ntff_profile_hook(
            _ntff_profile_via_ctypes("/opt/axon/libaxon_pjrt.so"))
except Exception:
    pass

import numpy as np
import ml_dtypes

import concourse.bass as bass
import concourse.bacc as bacc
import concourse.mybir as mybir
import concourse.tile as tile
from concourse.bass_utils import run_bass_kernel_spmd

BF16 = ml_dtypes.bfloat16
F32 = mybir.dt.float32
BF = mybir.dt.bfloat16
AL = mybir.AluOpType
AF = mybir.ActivationFunctionType
AX = mybir.AxisListType

B, N, D, H = 4, 16384, 256, 2
C = D // H
T = N // 2                 # tokens per core
NT = T // 128              # 64 token tiles / core
EPS_LN = 1e-5
EPS_NORM = 1e-12
N_CORES = 8
CHT = 8                    # token tiles per chunk
NCH = NT // CHT            # 8 chunks
PAYW = 260                 # collective payload width (G 256 | dq 2 | dk 2)

_nc_cache = {}


def _build_nc():
    nc = bacc.Bacc("TRN2", target_bir_lowering=False, debug=False,
                   num_devices=N_CORES)

    def ein(name, shape, dt=F32):
        return nc.dram_tensor(name, list(shape), dt, kind="ExternalInput")

    d_s = ein("x_s", [T, D])            # q source shard (input_S)
    d_r = ein("x_r", [T, D])            # kv source shard (input_R)
    d_wqT = ein("wqT", [D, D], BF)      # Wq'(=Wq.diag(lnS_w)) transposed [e,c]
    d_wkT = ein("wkT", [D, D], BF)
    d_wv = ein("wv", [D, D], BF)        # Wv' natural [c, e]
    d_woT = ein("woT", [D, D], BF)      # Wo transposed [c, o]
    d_rows = ein("rows", [1, 6 * D], BF)  # bq|bk|2bq|2bk|T*bq|T*bk rows
    d_bv = ein("bv_col", [128, H], BF)
    d_bo = ein("bo_col", [128, H])
    d_temp = ein("temp_col", [128, H])
    d_eyef = ein("eyef", [128, 128])
    d_eyeb = ein("eyeb", [128, 128], BF)
    d_out = nc.dram_tensor("out", [T, D], BF, kind="ExternalOutput")

    sv = d_s.rearrange("(j p) d -> p j d", p=128)
    rv = d_r.rearrange("(j p) d -> p j d", p=128)
    outv = d_out.rearrange("(j p) d -> p j d", p=128)

    with tile.TileContext(nc) as tc:
        import contextlib
        with contextlib.ExitStack() as ctx:
            _body(ctx, tc, nc, sv, rv, outv, d_wqT, d_wkT, d_wv, d_woT,
                  d_rows, d_bv, d_bo, d_temp, d_eyef, d_eyeb)
    nc.finalize()
    return nc


def _body(ctx, tc, nc, sv, rv, outv, d_wqT, d_wkT, d_wv, d_woT, d_rows,
          d_bv, d_bo, d_temp, d_eyef, d_eyeb):
    E = ctx.enter_context
    consts = E(tc.tile_pool(name="consts", bufs=1))
    stage = E(tc.tile_pool(name="stage", bufs=2))
    sqp = E(tc.tile_pool(name="sqp", bufs=2))
    nrm = E(tc.tile_pool(name="nrm", bufs=2))
    stp = E(tc.tile_pool(name="stp", bufs=2))
    pers = E(tc.tile_pool(name="pers", bufs=1))
    post = E(tc.tile_pool(name="post", bufs=1))
    small = E(tc.tile_pool(name="small", bufs=4))
    outp = E(tc.tile_pool(name="outp", bufs=2))
    dram = E(tc.tile_pool(name="dram", bufs=1, space="DRAM"))
    gacc = E(tc.tile_pool(name="gacc", bufs=1, space="PSUM"))

    # ---------------- constants ----------------
    wqT = consts.tile([128, 2, D], BF, tag="wqT")
    wkT = consts.tile([128, 2, D], BF, tag="wkT")
    wv_sb = consts.tile([128, 2, D], BF, tag="wv")
    woT = consts.tile([128, 2, D], BF, tag="woT")
    for dst, src in ((wqT, d_wqT), (wkT, d_wkT), (wv_sb, d_wv), (woT, d_woT)):
        nc.sync.dma_start(out=dst[:], in_=src.rearrange("(h p) o -> p h o", p=128))
    rows_sb = consts.tile([1, 6 * D], BF, tag="rows")
    nc.sync.dma_start(out=rows_sb[:], in_=d_rows[:, :])
    bq_row = rows_sb[0:1, 0 * D:1 * D]
    bk_row = rows_sb[0:1, 1 * D:2 * D]
    bq2_row = rows_sb[0:1, 2 * D:3 * D]
    bk2_row = rows_sb[0:1, 3 * D:4 * D]
    bqT_row = rows_sb[0:1, 4 * D:5 * D]
    bkT_row = rows_sb[0:1, 5 * D:6 * D]
    bv_col = consts.tile([128, H], BF, tag="bv")
    bo_col = consts.tile([128, H], F32, tag="bo")
    temp_col = consts.tile([128, H], F32, tag="temp")
    for dst, src in ((bv_col, d_bv), (bo_col, d_bo), (temp_col, d_temp)):
        nc.sync.dma_start(out=dst[:], in_=src[:, :])
    eyef = consts.tile([128, 128], F32, tag="eyef")
    eyeb = consts.tile([128, 128], BF, tag="eyeb")
    nc.sync.dma_start(out=eyef[:], in_=d_eyef[:, :])
    nc.sync.dma_start(out=eyeb[:], in_=d_eyeb[:, :])
    ones_col = consts.tile([128, 1], BF, tag="ones_c")
    nc.vector.memset(ones_col[:], 1.0)
    ones_row = consts.tile([1, 128], BF, tag="ones_r")
    nc.gpsimd.memset(ones_row[:], 1.0)
    epsln = consts.tile([128, 1], F32, tag="epsln")
    nc.vector.memset(epsln[:], EPS_LN)
    zcol = consts.tile([128, 1], F32, tag="zcol")
    nc.vector.memset(zcol[:], 0.0)

    rdm = pers.tile([128, NT, 2, 128], BF, tag="rdm")   # normalized R, d-major

    psSS = gacc.tile([128, 2, 256], F32, tag="psSS")
    psRX = gacc.tile([128, 2, 512], F32, tag="psRX")    # [M_RS | M_RR] blocks
    psSum = gacc.tile([128, 512], F32, tag="psSum")     # row 0: [s_S | s_R]

    # ================= phase 1: stream chunks =================
    for ch in range(NCH):
        j0 = ch * CHT
        raw = stage.tile([128, 2, CHT, 256], BF, tag="raw")  # 0=S 1=R
        nc.gpsimd.dma_start(out=raw[:, 0], in_=sv[:, j0:j0 + CHT, :])
        nc.gpsimd.dma_start(out=raw[:, 1], in_=rv[:, j0:j0 + CHT, :])

        sq = sqp.tile([128, 2, CHT, 256], BF, tag="sq")
        nc.scalar.activation(out=sq[:], in_=raw[:], func=AF.Square,
                             bias=zcol[:, :], scale=1.0)
        s1 = stp.tile([128, 2, CHT], F32, tag="s1")
        s2 = stp.tile([128, 2, CHT], F32, tag="s2")
        nc.vector.tensor_reduce(out=s1[:], in_=raw[:], axis=AX.X, op=AL.add)
        nc.vector.tensor_reduce(out=s2[:], in_=sq[:], axis=AX.X, op=AL.add)

        mu = stp.tile([128, 2, CHT], F32, tag="mu")
        var = stp.tile([128, 2, CHT], F32, tag="var")
        sig = stp.tile([128, 2, CHT], F32, tag="sig")
        a_sc = stp.tile([128, 2, CHT], F32, tag="a_sc")
        b_sc = stp.tile([128, 2, CHT], F32, tag="b_sc")
        nc.vector.tensor_scalar(mu[:], s1[:], 1.0 / D, None, AL.mult)
        nc.vector.scalar_tensor_tensor(out=var[:], in0=mu[:], scalar=-1.0,
                                       op0=AL.mult, op1=AL.mult, in1=mu[:])
        nc.vector.scalar_tensor_tensor(out=var[:], in0=s2[:], scalar=1.0 / D,
                                       op0=AL.mult, op1=AL.add, in1=var[:])
        nc.scalar.activation(out=sig[:], in_=var[:], func=AF.Sqrt,
                             bias=epsln[:, :], scale=1.0)
        nc.vector.reciprocal(out=a_sc[:], in_=sig[:])
        nc.vector.scalar_tensor_tensor(out=b_sc[:], in0=mu[:], scalar=-1.0,
                                       op0=AL.mult, op1=AL.mult, in1=a_sc[:])

        nt = nrm.tile([128, 2, CHT, 256], BF, tag="nt")
        for jj in range(CHT):
            nc.vector.tensor_scalar(nt[:, 0, jj], raw[:, 0, jj],
                                    a_sc[:, 0, jj:jj + 1],
                                    b_sc[:, 0, jj:jj + 1], AL.mult, AL.add)
            nc.scalar.activation(out=nt[:, 1, jj], in_=raw[:, 1, jj],
                                 func=AF.Identity, bias=b_sc[:, 1, jj:jj + 1],
                                 scale=a_sc[:, 1, jj:jj + 1])

        for jj in range(CHT):
            j = j0 + jj
            st = (j == 0)
            sp = (j == NT - 1)
            for ech in range(2):
                nc.tensor.matmul(out=psSS[:, ech, :],
                                 lhsT=nt[:, 0, jj, ech * 128:(ech + 1) * 128],
                                 rhs=nt[:, 0, jj, :], start=st, stop=sp)
            for fch in range(2):
                nc.tensor.matmul(out=psRX[:, fch, :],
                                 lhsT=nt[:, 1, jj, fch * 128:(fch + 1) * 128],
                                 rhs=nt[:, :, jj, :], start=st, stop=sp)
            nc.tensor.matmul(out=psSum[0:1, :], lhsT=ones_col[:],
                             rhs=nt[:, :, jj, :], start=st, stop=sp)

        nc.sync.dma_start_transpose(rdm[:, j0:j0 + CHT, :, :], nt[:, 1])

    # ================= phase 2: local reductions -> payload ================
    mSS = post.tile([128, 2, 256], BF, tag="mSS")
    mRS = post.tile([128, 2, 256], BF, tag="mRS")
    mRR = post.tile([128, 2, 256], BF, tag="mRR")
    nc.vector.tensor_scalar(mSS[:], psSS[:], 1.0, None, AL.mult)
    nc.scalar.activation(out=mRS[:], in_=psRX[:, :, 0:256], func=AF.Identity,
                         bias=zcol[:, :], scale=1.0)
    nc.scalar.activation(out=mRR[:], in_=psRX[:, :, 256:512], func=AF.Identity,
                         bias=zcol[:, :], scale=1.0)
    sums_sb = post.tile([1, 512], F32, tag="sums")
    nc.vector.tensor_scalar(sums_sb[:], psSum[0:1, :], 1.0, None, AL.mult)

    with tc.tile_pool(name="pps", bufs=2, space="PSUM") as pps:
        # s_S / s_R rows -> bf16 columns via PE transpose
        ps_sc = pps.tile([128, 2, 512], F32, tag="ps")
        for k in range(4):
            nc.tensor.transpose(ps_sc[:, 0, k:k + 1],
                                sums_sb[0:1, k * 128:(k + 1) * 128],
                                eyef[0:1, 0:1])
        scol = post.tile([128, 4], BF, tag="scol")   # sS e0,e1 | sR e0,e1
        nc.vector.tensor_scalar(scol[:], ps_sc[:, 0, 0:4], 1.0, None, AL.mult)

        # (Wq' s_S) and (Wk' s_R) as rows [1, 256]
        ps_r = pps.tile([128, 2, 512], F32, tag="ps")
        for ech in range(2):
            nc.tensor.matmul(out=ps_r[0:1, 0, 0:256], lhsT=scol[:, ech:ech + 1],
                             rhs=wqT[:, ech, :], start=(ech == 0),
                             stop=(ech == 1))
            nc.tensor.matmul(out=ps_r[0:1, 0, 256:512],
                             lhsT=scol[:, 2 + ech:3 + ech],
                             rhs=wkT[:, ech, :], start=(ech == 0),
                             stop=(ech == 1))
        prows = post.tile([1, 512], BF, tag="prows")  # wqss row | wksr row
        nc.vector.tensor_scalar(prows[:], ps_r[0:1, 0, :], 1.0, None, AL.mult)
        wqss_row = prows[0:1, 0:256]
        wksr_row = prows[0:1, 256:512]

        # V = M_SR Wk'^T  (lhsT = M_RS blocks)
        ps_v = pps.tile([128, 2, 512], F32, tag="ps")
        for ech in range(2):
            for fch in range(2):
                nc.tensor.matmul(out=ps_v[:, ech, 0:256],
                                 lhsT=mRS[:, fch, ech * 128:(ech + 1) * 128],
                                 rhs=wkT[:, fch, :], start=(fch == 0),
                                 stop=(fch == 1))
        v_sb = post.tile([128, 2, 256], BF, tag="v_sb")
        nc.scalar.activation(out=v_sb[:], in_=ps_v[:, :, 0:256],
                             func=AF.Identity, bias=zcol[:, :], scale=1.0)

        # Z_q = M_SS Wq'^T-ish, Z_k = M_RR Wk'^T (for norm diagonals)
        ps_z = pps.tile([128, 2, 512], F32, tag="ps")
        for ech in range(2):
            for fch in range(2):
                nc.tensor.matmul(out=ps_z[:, ech, 0:256],
                                 lhsT=mSS[:, fch, ech * 128:(ech + 1) * 128],
                                 rhs=wqT[:, fch, :], start=(fch == 0),
                                 stop=(fch == 1))
                nc.tensor.matmul(out=ps_z[:, ech, 256:512],
                                 lhsT=mRR[:, fch, ech * 128:(ech + 1) * 128],
                                 rhs=wkT[:, fch, :], start=(fch == 0),
                                 stop=(fch == 1))
        z_sb = post.tile([128, 2, 512], BF, tag="z_sb")
        nc.vector.tensor_scalar(z_sb[:], ps_z[:], 1.0, None, AL.mult)

        payload = post.tile([128, PAYW], F32, tag="payload")

        # G head blocks + bias outers
        ps_g = pps.tile([128, 2, 512], F32, tag="ps")
        for hh in range(2):
            g = ps_g[:, 0, hh * 128:(hh + 1) * 128]
            hs = slice(hh * 128, (hh + 1) * 128)
            for ech in range(2):
                nc.tensor.matmul(out=g, lhsT=wqT[:, ech, hs],
                                 rhs=v_sb[:, ech, hs], start=(ech == 0),
                                 stop=False)
            nc.tensor.matmul(out=g, lhsT=bq_row[:, hs], rhs=wksr_row[:, hs],
                             start=False, stop=False)
            nc.tensor.matmul(out=g, lhsT=wqss_row[:, hs], rhs=bk_row[:, hs],
                             start=False, stop=False)
            nc.tensor.matmul(out=g, lhsT=bq_row[:, hs], rhs=bkT_row[:, hs],
                             start=False, stop=True)
        nc.vector.tensor_scalar(payload[:, 0:256], ps_g[:, 0, 0:256], 1.0,
                                None, AL.mult)

        # Y_q / Y_k head blocks + bias outers; eye-dot -> dq, dk
        ps_y = pps.tile([128, 2, 512], F32, tag="ps")
        for hh in range(2):
            hs = slice(hh * 128, (hh + 1) * 128)
            yq = ps_y[:, hh, 0:128]
            yk = ps_y[:, hh, 128:256]
            for ech in range(2):
                nc.tensor.matmul(out=yq, lhsT=wqT[:, ech, hs],
                                 rhs=z_sb[:, ech, hs], start=(ech == 0),
                                 stop=False)
            nc.tensor.matmul(out=yq, lhsT=bq2_row[:, hs], rhs=wqss_row[:, hs],
                             start=False, stop=False)
            nc.tensor.matmul(out=yq, lhsT=bq_row[:, hs], rhs=bqT_row[:, hs],
                             start=False, stop=True)
            for ech in range(2):
                nc.tensor.matmul(
                    out=yk, lhsT=wkT[:, ech, hs],
                    rhs=z_sb[:, ech, 256 + hh * 128:256 + (hh + 1) * 128],
                    start=(ech == 0), stop=False)
            nc.tensor.matmul(out=yk, lhsT=bk2_row[:, hs], rhs=wksr_row[:, hs],
                             start=False, stop=False)
            nc.tensor.matmul(out=yk, lhsT=bk_row[:, hs], rhs=bkT_row[:, hs],
                             start=False, stop=True)
            dscr = small.tile([128, 128], F32, tag="dscr")
            nc.vector.scalar_tensor_tensor(
                out=dscr[:], in0=yq, scalar=1.0, op0=AL.mult, op1=AL.mult,
                in1=eyef[:], accum_out=payload[:, 256 + hh:257 + hh])
            nc.vector.scalar_tensor_tensor(
                out=dscr[:], in0=yk, scalar=1.0, op0=AL.mult, op1=AL.mult,
                in1=eyef[:], accum_out=payload[:, 258 + hh:259 + hh])

        # ---------------- collective ----------------
        cc_in = dram.tile([128, PAYW], F32)
        cc_out = dram.tile([128, PAYW], F32)
        nc.gpsimd.dma_start(out=cc_in[:, :], in_=payload[:])
        nc.gpsimd.collective_compute(
            "AllReduce", AL.add,
            replica_groups=[[0, 1], [2, 3], [4, 5], [6, 7]],
            ins=[cc_in.opt()], outs=[cc_out.opt()])
        red = post.tile([128, PAYW], F32, tag="red")
        nc.sync.dma_start(out=red[:], in_=cc_out[:, :])

        # ---------------- phase 3: softmax + W_eff ----------------
        nrmc = small.tile([128, 4], F32, tag="nrmc")
        nc.scalar.activation(out=nrmc[:], in_=red[:, 256:260], func=AF.Sqrt,
                             bias=zcol[:, :], scale=1.0)
        nc.vector.tensor_scalar_max(nrmc[:], nrmc[:], EPS_NORM)
        nc.vector.reciprocal(out=nrmc[:], in_=nrmc[:])
        iq = small.tile([128, 2], F32, tag="iq")
        nc.vector.tensor_tensor(out=iq[:], in0=nrmc[:, 0:2], in1=temp_col[:],
                                op=AL.mult)

        # invk column -> broadcast tile via PE transpose + outer product
        ps_t = pps.tile([128, 2, 512], F32, tag="ps")
        ikrow = post.tile([1, 2, 128], BF, tag="ikrow")
        for hh in range(2):
            nc.tensor.transpose(ps_t[0:1, 0, hh * 128:(hh + 1) * 128],
                                nrmc[:, 2 + hh:3 + hh], eyef[:])
            nc.vector.tensor_scalar(
                ikrow[0:1, hh, :], ps_t[0:1, 0, hh * 128:(hh + 1) * 128],
                1.0, None, AL.mult)
        ps_ik = pps.tile([128, 2, 512], F32, tag="ps")
        for hh in range(2):
            nc.tensor.matmul(out=ps_ik[:, 0, hh * 128:(hh + 1) * 128],
                             lhsT=ones_row[:], rhs=ikrow[0:1, hh, :],
                             start=True, stop=True)
        ikb = post.tile([128, 2, 128], F32, tag="ikb")
        nc.vector.tensor_scalar(ikb[:], ps_ik[:, 0, 0:256], 1.0, None, AL.mult)

        # logits, softmax
        lg = post.tile([128, 2, 128], F32, tag="lg")
        for hh in range(2):
            nc.vector.tensor_scalar(lg[:, hh, :], red[:, hh * 128:(hh + 1) * 128],
                                    iq[:, hh:hh + 1], None, AL.mult)
        nc.vector.tensor_tensor(out=lg[:], in0=lg[:], in1=ikb[:], op=AL.mult)
        rmax = small.tile([128, 2], F32, tag="rmax")
        nc.vector.tensor_reduce(out=rmax[:], in_=lg[:], axis=AX.X, op=AL.max)
        nc.vector.tensor_scalar(rmax[:], rmax[:], -1.0, None, AL.mult)
        att = post.tile([128, 2, 128], F32, tag="att")
        for hh in range(2):
            nc.scalar.activation(out=att[:, hh, :], in_=lg[:, hh, :],
                                 func=AF.Exp, bias=rmax[:, hh:hh + 1],
                                 scale=1.0)
        rs = small.tile([128, 2], F32, tag="rs")
        nc.vector.tensor_reduce(out=rs[:], in_=att[:], axis=AX.X, op=AL.add)
        nc.vector.reciprocal(out=rs[:], in_=rs[:])
        attf = post.tile([128, 2, 128], F32, tag="attf")
        for hh in range(2):
            nc.vector.tensor_scalar(attf[:, hh, :], att[:, hh, :],
                                    rs[:, hh:hh + 1], None, AL.mult)

        # attn^T
        ps_at = pps.tile([128, 2, 512], F32, tag="ps")
        for hh in range(2):
            nc.tensor.transpose(ps_at[:, hh, 0:128], attf[:, hh, :], eyef[:])
        attT = post.tile([128, 2, 128], BF, tag="attT")
        nc.vector.tensor_scalar(attT[:], ps_at[:, :, 0:128], 1.0, None,
                                AL.mult)

        # A1_h = attn_h @ Wv'_h ; W_effT ; f2
        ps_a1 = pps.tile([128, 2, 512], F32, tag="ps")
        for hh in range(2):
            nc.tensor.matmul(out=ps_a1[:, hh, 0:256], lhsT=attT[:, hh, :],
                             rhs=wv_sb[:, hh, :], start=True, stop=True)
            nc.tensor.matmul(out=ps_a1[:, hh, 256:257], lhsT=attT[:, hh, :],
                             rhs=bv_col[:, hh:hh + 1], start=True, stop=True)
        a1 = post.tile([128, 2, 256], BF, tag="a1")
        nc.scalar.activation(out=a1[:], in_=ps_a1[:, :, 0:256],
                             func=AF.Identity, bias=zcol[:, :], scale=1.0)
        rc = small.tile([128, 2], BF, tag="rc")
        nc.vector.tensor_scalar(rc[:], ps_a1[:, :, 256], 1.0, None, AL.mult)

        ps_we = pps.tile([128, 2, 512], F32, tag="ps")
        for dch in range(2):
            for hh in range(2):
                nc.tensor.matmul(out=ps_we[:, dch, 0:256],
                                 lhsT=a1[:, hh, dch * 128:(dch + 1) * 128],
                                 rhs=woT[:, hh, :], start=(hh == 0),
                                 stop=(hh == 1))
        weT = post.tile([128, 2, 256], BF, tag="weT")
        nc.scalar.activation(out=weT[:], in_=ps_we[:, :, 0:256],
                             func=AF.Identity, bias=zcol[:, :], scale=1.0)

        # f2 = Wo rc + bo (column), -> row -> broadcast tile
        ps_f2 = pps.tile([128, 2, 512], F32, tag="ps")
        for och in range(2):
            for hh in range(2):
                nc.tensor.matmul(
                    out=ps_f2[:, 0, och:och + 1],
                    lhsT=woT[:, hh, och * 128:(och + 1) * 128],
                    rhs=rc[:, hh:hh + 1], start=(hh == 0), stop=(hh == 1))
        f2c = small.tile([128, 2], F32, tag="f2c")
        nc.vector.scalar_tensor_tensor(out=f2c[:], in0=ps_f2[:, 0, 0:2],
                                       scalar=1.0, op0=AL.bypass, op1=AL.add,
                                       in1=bo_col[:])
        ps_f2r = pps.tile([128, 2, 512], F32, tag="ps")
        f2row = post.tile([1, 2, 128], BF, tag="f2row")
        for och in range(2):
            nc.tensor.transpose(ps_f2r[0:1, 0, och * 128:(och + 1) * 128],
                                f2c[:, och:och + 1], eyef[:])
            nc.vector.tensor_scalar(
                f2row[0:1, och, :],
                ps_f2r[0:1, 0, och * 128:(och + 1) * 128], 1.0, None, AL.mult)
        ps_fb = pps.tile([128, 2, 512], F32, tag="ps")
        for och in range(2):
            nc.tensor.matmul(out=ps_fb[:, 0, och * 128:(och + 1) * 128],
                             lhsT=ones_row[:], rhs=f2row[0:1, och, :],
                             start=True, stop=True)
        f2b = post.tile([128, 256], F32, tag="f2b")
        nc.vector.tensor_scalar(f2b[:], ps_fb[:, 0, 0:256], 1.0, None, AL.mult)

    # ================= phase 4: output pass =================
    with tc.tile_pool(name="ops", bufs=4, space="PSUM") as ops:
        for g in range(NT // 4):
            osb = outp.tile([128, 4, 256], BF, tag="osb")
            for q4 in range(4):
                j = g * 4 + q4
                op_ps = ops.tile([128, 256], F32, tag="op")
                for dch in range(2):
                    nc.tensor.matmul(out=op_ps[:], lhsT=rdm[:, j, dch, :],
                                     rhs=weT[:, dch, :], start=(dch == 0),
                                     stop=(dch == 1))
                nc.vector.scalar_tensor_tensor(
                    out=osb[:, q4, :], in0=op_ps[:], scalar=1.0,
                    op0=AL.bypass, op1=AL.add, in1=f2b[:])
            nc.sync.dma_start(out=outv[:, g * 4:(g + 1) * 4, :], in_=osb[:])


# ======================= host side =======================

def _prep_shared(inputs):
    f32 = np.float32
    Wq = np.asarray(inputs["Wq"], f32)
    bq = np.asarray(inputs["bq"], f32)
    Wkv = np.asarray(inputs["Wkv"], f32)
    bkv = np.asarray(inputs["bkv"], f32)
    Wo = np.asarray(inputs["Wo"], f32)
    bo = np.asarray(inputs["bo"], f32)
    lnS_w = np.asarray(inputs["lnS_w"], f32)
    lnS_b = np.asarray(inputs["lnS_b"], f32)
    lnR_w = np.asarray(inputs["lnR_w"], f32)
    lnR_b = np.asarray(inputs["lnR_b"], f32)
    temp = np.asarray(inputs["temperature"], f32).reshape(H)

    Wk, Wv = Wkv[:D], Wkv[D:]
    Wqp = Wq * lnS_w[None, :]
    Wkp = Wk * lnR_w[None, :]
    Wvp = Wv * lnR_w[None, :]
    bq2 = Wq @ lnS_b + bq
    bk2 = Wk @ lnR_b + bkv[:D]
    bv2 = Wv @ lnR_b + bkv[D:]

    def colh(v, dt=f32):
        return np.ascontiguousarray(v.reshape(H, 128).T).astype(dt)

    rows = np.concatenate([bq2, bk2, 2.0 * bq2, 2.0 * bk2,
                           float(T) * bq2, float(T) * bk2]).reshape(1, 6 * D)
    return {
        "wqT": np.ascontiguousarray(Wqp.T).astype(BF16),
        "wkT": np.ascontiguousarray(Wkp.T).astype(BF16),
        "wv": np.ascontiguousarray(Wvp).astype(BF16),
        "woT": np.ascontiguousarray(Wo.T).astype(BF16),
        "rows": rows.astype(BF16),
        "bv_col": colh(bv2, BF16),
        "bo_col": colh(bo),
        "temp_col": np.broadcast_to(temp[None, :], (128, H)).astype(f32).copy(),
        "eyef": np.eye(128, dtype=f32),
        "eyeb": np.eye(128, dtype=f32).astype(BF16),
    }


def _get_nc():
    if "nc" not in _nc_cache:
        _nc_cache["nc"] = _build_nc()
    return _nc_cache["nc"]


def run(inputs, trace=False):
    nc = _get_nc()
    shared = _prep_shared(inputs)
    iR = np.asarray(inputs["input_R"], np.float32)
    iS = np.asarray(inputs["input_S"], np.float32)
    in_maps = []
    for ci in range(N_CORES):
        b, half = ci // 2, ci % 2
        m = dict(shared)
        m["x_r"] = np.ascontiguousarray(iR[b, half * T:(half + 1) * T])
        m["x_s"] = np.ascontiguousarray(iS[b, half * T:(half + 1) * T])
        in_maps.append(m)
    res = run_bass_kernel_spmd(nc, in_maps, list(range(N_CORES)), trace=trace)
    out = np.zeros((B, N, D), np.float32)
    for ci in range(N_CORES):
        b, half = ci // 2, ci % 2
        out[b, half * T:(half + 1) * T] = np.asarray(
            res.results[ci]["out"]).astype(np.float32)
    return out, res


def kernel(**inputs):
    out, _ = run(inputs, trace=False)
    return out
